# revision 13
# baseline (speedup 1.0000x reference)
"""Bass/Trainium2 kernel for nn_DynamicEdgeWeights.

Math (B=4, S=512, D=128, H=8):
    a = x @ w1[:D]; c = x @ w1[D:]
    h[b,i,j,:] = relu(a[b,i,:] + c[b,j,:] + b1)
    out[b,h,i,j] = sigmoid(sum_d h[b,i,j,d] * w2[d,h] + b2[h])

Device strategy (per core; 8 cores, core k -> batch k//2, i-rows [(k%2)*256, +256)):
  - cT[d, j] = (x[b] @ w1c).T and aT[d, i] = (x[b] @ w1a).T + b1 via two PE
    matmuls on pre-transposed x (host passes x[b].T).
  - per query row i: one fused relu(cT + aT[:, i]) producing h_i [128d, 512j]
    (DVE tensor_scalar add+max, or ACT activation Relu with per-partition bias).
  - second matmul uses "comb" weights: 16 query rows share one PSUM bank.
    comb_g [128, 128] has w2[:, h] in column h*16+g, zeros elsewhere; 16
    accumulating matmuls put e-pre for (16 i x 8 h) on 128 PSUM partitions.
  - groups are processed in pairs sharing a 2-bank PSUM tile; one full-width
    [128, 1024] sigmoid (ACT, bias=b2 broadcast) -> SBUF -> stores split
    across both HWDGE queues straight into out[b, :, i-rows, :].
"""

import os
import sys

for _p in ("/opt/trn_rl_repo", "/root/.axon_site/_ro/trn_rl_repo"):
    if os.path.isdir(_p) and _p not in sys.path:
        sys.path.insert(0, _p)
        break

import numpy as np
import ml_dtypes  # noqa: F401  (registers bfloat16 dtype)

import concourse.bass as bass  # noqa: F401  (registers types)
import concourse.mybir as mybir
from concourse import bacc
from concourse.tile import TileContext

B, S, D, H = 4, 512, 128, 8
N_CORES = 8
I_PER_CORE = (B * S) // N_CORES  # 256
G = 16  # query rows packed per PSUM bank
T = I_PER_CORE // G  # 16 groups per core
# ACT h-gen share: values >= 8 mean (n_act - 8) of the last 8 sched entries
# per pair go to ScalarE (rest DVE); 13 -> 5 of 32 rows per pair on ACT
N_ACT = 13

F32 = mybir.dt.float32
F16 = mybir.dt.float16  # h-path dtype: full PE rate (fp32 streams at 1/4 rate)
F16_NP = "float16"

_CACHE: dict = {}


def _build_nc(loop_iters: int = 1, dt_h=F16, n_act=N_ACT, h_bufs=8, o_bufs=8, mm_bufs=4, staggered=False, diag=None, store_eng="big", mm_order="rqu"):
    """Build the single-core Bass program (identical across the 8 cores).

    loop_iters > 1 wraps the whole compute in an on-device For_i repeat —
    used only for steady-state timing (one dispatch, N executions).
    """
    nc = bacc.Bacc(
        "TRN2",
        target_bir_lowering=False,
        debug=False,
        enable_asserts=False,
        num_devices=N_CORES,
    )

    xTj_d = nc.dram_tensor("xTj", (D, S), F32, kind="ExternalInput").ap()
    xTi_d = nc.dram_tensor("xTi", (D, I_PER_CORE), F32, kind="ExternalInput").ap()
    w1a_d = nc.dram_tensor("w1a", (D, D), F32, kind="ExternalInput").ap()
    w1c_d = nc.dram_tensor("w1c", (D, D), F32, kind="ExternalInput").ap()
    b1_d = nc.dram_tensor("b1c", (D, 1), F32, kind="ExternalInput").ap()
    comb_d = nc.dram_tensor("comb", (D, 4 * 32), dt_h, kind="ExternalInput").ap()
    b2v_d = nc.dram_tensor("b2v", (D, 1), F32, kind="ExternalInput").ap()
    out_d = nc.dram_tensor("out", (H, I_PER_CORE, S), F32, kind="ExternalOutput").ap()

    relu = mybir.ActivationFunctionType.Relu
    sigmoid = mybir.ActivationFunctionType.Sigmoid
    add = mybir.AluOpType.add
    amax = mybir.AluOpType.max

    import contextlib

    with TileContext(nc) as tc:
        with (
            tc.tile_pool(name="const", bufs=1) as cpool,
            tc.tile_pool(name="h", bufs=h_bufs) as hpool,
            tc.tile_pool(name="o", bufs=o_bufs) as opool,
            tc.tile_pool(name="mm", bufs=mm_bufs, space="PSUM") as mmpool,
            (
                tc.For_i(
                    0,
                    loop_iters,
                    1,
                    hint_engines=(
                        mybir.EngineType.PE,
                        mybir.EngineType.DVE,
                        mybir.EngineType.Activation,
                        mybir.EngineType.SP,
                    ),
                    staggered_reset=staggered,
                )
                if loop_iters > 1
                else contextlib.nullcontext()
            ),
        ):
            xj_sb = cpool.tile([D, S], F32)
            nc.sync.dma_start(out=xj_sb, in_=xTj_d)
            xi_sb = cpool.tile([D, I_PER_CORE], F32)
            nc.sync.dma_start(out=xi_sb, in_=xTi_d)
            w1a_sb = cpool.tile([D, D], F32)
            nc.sync.dma_start(out=w1a_sb, in_=w1a_d)
            w1c_sb = cpool.tile([D, D], F32)
            nc.sync.dma_start(out=w1c_sb, in_=w1c_d)
            comb_sb = cpool.tile([D, 4 * 32], dt_h)
            nc.scalar.dma_start(out=comb_sb, in_=comb_d)
            b1_sb = cpool.tile([D, 1], F32)
            nc.sync.dma_start(out=b1_sb, in_=b1_d)
            b2v_sb = cpool.tile([D, 1], F32)
            nc.sync.dma_start(out=b2v_sb, in_=b2v_d)

            # precompute borrows one pair-slot from the matmul psum pool:
            # cT in the first bank-half, aT in the second
            pre_ps = mmpool.tile([D, 2 * S], F32, tag="ps2")
            # cT[d_out, j] = sum_k w1c[k, d_out] * xT[k, j]
            nc.tensor.matmul(pre_ps[:, :S], w1c_sb, xj_sb, start=True, stop=True)
            cT_sb = cpool.tile([D, S], dt_h)
            nc.vector.tensor_copy(cT_sb, pre_ps[:, :S])

            # aT[d_out, i] = sum_k w1a[k, d_out] * xT[k, i]  (+ b1 per partition)
            nc.tensor.matmul(
                pre_ps[:, S : S + I_PER_CORE], w1a_sb, xi_sb, start=True, stop=True
            )
            at_sb = cpool.tile([D, I_PER_CORE], F32)
            nc.vector.tensor_scalar_add(at_sb, pre_ps[:, S : S + I_PER_CORE], b1_sb)

            def drain(t, ps2):
                # sigmoid + store for a finished pair of groups (t, t+1);
                # emitted one pair late so ACT's (stalling) sigmoid sits
                # behind the next pair's h-gen ops in ACT program order.
                o_sb = opool.tile([D, 2 * S], F32)
                nc.scalar.activation(o_sb, ps2, sigmoid, bias=b2v_sb)
                # partition p = g*8+h  ->  out[h, (t+u)*16+g, :]; all four
                # stores issue from SP's HWDGE (ACT stays compute-only)
                half = D // 2
                for u in range(2):
                    dst = out_d[:, (t + u) * G : (t + u + 1) * G, :].rearrange(
                        "h g j -> g h j"
                    )
                    src = o_sb[:, u * S : (u + 1) * S]
                    if store_eng == "big":
                        # one full-width store per u-half; alternate engines
                        eng = nc.sync if u == 0 else nc.scalar
                        eng.dma_start(out=dst, in_=src)
                    elif store_eng == "bigsp":
                        nc.sync.dma_start(out=dst, in_=src)
                    else:  # "split": halves across SP + ACT queues
                        nc.sync.dma_start(out=dst[: G // 2], in_=src[:half])
                        nc.scalar.dma_start(out=dst[G // 2 :], in_=src[half:])

            if diag == "pe":
                # PE-pure stream: one static h tile, full matmul schedule
                h_static = cpool.tile([D, 2 * S], dt_h)
                nc.vector.tensor_copy(h_static[:, :S], cT_sb)
                nc.vector.tensor_copy(h_static[:, S:], cT_sb)
                for t in range(0, T, 2):
                    ps2 = mmpool.tile([D, 2 * S], F32, tag="ps2")
                    g_order = [4 * q + r for r in range(4) for q in range(4)]
                    for n, g in enumerate(g_order):
                        q, r = g // 4, g % 4
                        for u in range(2):
                            nc.tensor.matmul(
                                ps2[32 * q : 32 * (q + 1), u * S : (u + 1) * S],
                                comb_sb[:, 32 * r : 32 * (r + 1)],
                                h_static[:, u * S : (u + 1) * S],
                                start=(r == 0),
                                stop=(r == 3),
                                tile_position=(0, 32 * q),
                                skip_group_check=True,
                            )
                    o_sb = opool.tile([D, 2 * S], F32)
                    nc.scalar.activation(o_sb, ps2, sigmoid, bias=b2v_sb)
                    half = D // 2
                    for u in range(2):
                        dst = out_d[:, (t + u) * G : (t + u + 1) * G, :]
                        src = o_sb[:, u * S : (u + 1) * S]
                        nc.sync.dma_start(out=dst[: H // 2], in_=src[:half])
                        nc.scalar.dma_start(out=dst[H // 2 :], in_=src[half:])
            elif diag == "dve":
                # DVE-pure stream: all h-gen ops, no matmul/sigmoid; dump one
                # h tile to out to keep outputs written
                for t in range(0, T, 2):
                    for g in range(G):
                        h2 = hpool.tile([D, 2 * S], dt_h)
                        for u in range(2):
                            i_loc = (t + u) * G + g
                            a_col = at_sb[:, i_loc : i_loc + 1]
                            dst = h2[:, u * S : (u + 1) * S]
                            nc.vector.tensor_scalar(dst, cT_sb, a_col, 0.0, add, amax)
                    o_sb = opool.tile([D, 2 * S], F32)
                    nc.vector.tensor_copy(o_sb, h2)
                    half = D // 2
                    for u in range(2):
                        dst = out_d[:, (t + u) * G : (t + u + 1) * G, :]
                        src = o_sb[:, u * S : (u + 1) * S]
                        nc.sync.dma_start(out=dst[: H // 2], in_=src[:half])
                        nc.scalar.dma_start(out=dst[H // 2 :], in_=src[half:])
            else:
                pending = None  # (t, psum tile) awaiting sigmoid+store
                # emission order: r outer, u middle, q fastest -> consecutive
                # matmuls land in 4 different 32-col PE strips (concurrent
                # streaming); a strip's accumulation chain (same q,u across r)
                # recurs only every 8 instructions.
                if mm_order == "ruq":
                    sched = [
                        (4 * q + r, u)
                        for r in range(4)
                        for u in range(2)
                        for q in range(4)
                    ]
                else:  # "rqu": the original order, u innermost
                    sched = [
                        (4 * q + r, u)
                        for r in range(4)
                        for q in range(4)
                        for u in range(2)
                    ]
                for t in range(0, T, 2):
                    # two groups (t, t+1) share one 2-bank PSUM tile: matmul g
                    # covers j 0..511 for row t*16+g and j 512..1023 for row
                    # (t+1)*16+g with the same comb_g weights.
                    ps2 = mmpool.tile([D, 2 * S], F32, tag="ps2")
                    for n, (g, u) in enumerate(sched):
                        q, r = g // 4, g % 4
                        i_loc = (t + u) * G + g
                        a_col = at_sb[:, i_loc : i_loc + 1]
                        hu = hpool.tile([D, S], dt_h, tag=f"h{u}")
                        # last act_k of the 32 sched entries go to ACT (all
                        # r==3 tail positions when act_k <= 8)
                        act_k = n_act * 2 if n_act < 8 else n_act - 8
                        if n >= 32 - act_k:
                            nc.scalar.activation(hu, cT_sb, relu, bias=a_col)
                        else:
                            nc.vector.tensor_scalar(hu, cT_sb, a_col, 0.0, add, amax)
                        nc.tensor.matmul(
                            ps2[32 * q : 32 * (q + 1), u * S : (u + 1) * S],
                            comb_sb[:, 32 * r : 32 * (r + 1)],
                            hu,
                            start=(r == 0),
                            stop=(r == 3),
                            tile_position=(0, 32 * q),
                            skip_group_check=True,
                        )
                        if n == 3 and pending is not None:
                            drain(*pending)
                            pending = None
                    pending = (t, ps2)
                drain(*pending)

    nc.compile()
    return nc


def _host_prep(node_features, w1, b1, w2, b2):
    """Shared (per-core-replicated) small tensors + per-core input maps."""
    w1a = np.ascontiguousarray(w1[:D])  # [D, D] == lhsT for aT
    w1c = np.ascontiguousarray(w1[D:])  # [D, D] == lhsT for cT
    b1c = np.ascontiguousarray(b1.reshape(D, 1))
    # psum partition p = g*8 + h; col-group q = g//4 covers partitions
    # [32q, 32q+32); weight tile r = g%4 has w2 in columns [8r, 8r+8)
    comb = np.zeros((D, 4, 32), np.float32)
    for r in range(4):
        comb[:, r, r * H : (r + 1) * H] = w2
    comb = np.ascontiguousarray(comb.reshape(D, 4 * 32).astype(F16_NP))
    b2v = np.ascontiguousarray(np.tile(b2, G).reshape(D, 1))

    in_maps = []
    for k in range(N_CORES):
        b = k // (N_CORES // B)
        i0 = (k % (N_CORES // B)) * I_PER_CORE
        xT = np.ascontiguousarray(node_features[b].T)  # [D, S]
        in_maps.append(
            {
                "xTj": xT,
                "xTi": np.ascontiguousarray(xT[:, i0 : i0 + I_PER_CORE]),
                "w1a": w1a,
                "w1c": w1c,
                "b1c": b1c,
                "comb": comb,
                "b2v": b2v,
            }
        )
    return in_maps


def _gather(results):
    out = np.empty((B, H, S, S), np.float32)
    for k in range(N_CORES):
        b = k // (N_CORES // B)
        i0 = (k % (N_CORES // B)) * I_PER_CORE
        out[b, :, i0 : i0 + I_PER_CORE, :] = results[k]["out"]
    return out


def _build_jit(nc):
    """Single cached jit around the bass_exec custom call (the stock
    run_bass_kernel_spmd path re-traces/jits on every invocation)."""
    import jax
    from jax.sharding import Mesh, PartitionSpec

    try:
        from jax.experimental.shard_map import shard_map
    except ImportError:
        from jax.sharding import shard_map

    from concourse.bass2jax import (
        _bass_exec_p,
        install_neuronx_cc_hook,
        partition_id_tensor,
    )

    install_neuronx_cc_hook()
    partition_name = nc.partition_id_tensor.name if nc.partition_id_tensor else None
    in_names, out_names, out_avals, zero_outs = [], [], [], []
    for alloc in nc.m.functions[0].allocations:
        if not isinstance(alloc, mybir.MemoryLocationSet):
            continue
        name = alloc.memorylocations[0].name
        if alloc.kind == "ExternalInput":
            if name != partition_name:
                in_names.append(name)
        elif alloc.kind == "ExternalOutput":
            shape = tuple(alloc.tensor_shape)
            np_dt = mybir.dt.np(alloc.dtype)
            out_avals.append(jax.core.ShapedArray(shape, np_dt))
            out_names.append(name)
            zero_outs.append(np.zeros(shape, np_dt))
    n_params = len(in_names)
    all_in_names = list(in_names) + list(out_names)
    if partition_name is not None:
        all_in_names.append(partition_name)

    def _body(*args):
        operands = list(args)
        if partition_name is not None:
            operands.append(partition_id_tensor())
        return tuple(
            _bass_exec_p.bind(
                *operands,
                out_avals=tuple(out_avals),
                in_names=tuple(all_in_names),
                out_names=tuple(out_names),
                lowering_input_output_aliases=(),
                sim_require_finite=True,
                sim_require_nnan=True,
                nc=nc,
            )
        )

    devices = jax.devices()[:N_CORES]
    mesh = Mesh(np.asarray(devices), ("core",))
    n_outs = len(out_names)
    sharded = jax.jit(
        shard_map(
            _body,
            mesh=mesh,
            in_specs=(PartitionSpec("core"),) * (n_params + n_outs),
            out_specs=(PartitionSpec("core"),) * n_outs,
            check_rep=False,
        ),
        # no donation: the kernel writes every output element, so the zero
        # operand buffers can live on device and be reused across calls
        keep_unused=True,
    )
    return sharded, in_names, out_names, zero_outs


def _run(in_maps):
    if "nc" not in _CACHE:
        _CACHE["nc"] = _build_nc()
        _CACHE["jit"] = _build_jit(_CACHE["nc"])
    sharded, in_names, out_names, zero_outs = _CACHE["jit"]
    concat_in = [
        np.concatenate([np.asarray(in_maps[c][n]) for c in range(N_CORES)], axis=0)
        for n in in_names
    ]
    if "zeros_dev" not in _CACHE:
        import jax

        _CACHE["zeros_dev"] = [
            jax.device_put(np.zeros((N_CORES * z.shape[0], *z.shape[1:]), z.dtype))
            for z in zero_outs
        ]
    out_arrs = sharded(*concat_in, *_CACHE["zeros_dev"])
    # outputs come back concatenated on axis 0 (N_CORES * dim0, ...)
    split = []
    for i, name in enumerate(out_names):
        arr = np.asarray(out_arrs[i])
        split.append(arr.reshape(N_CORES, arr.shape[0] // N_CORES, *arr.shape[1:]))
    return [
        {name: split[i][c] for i, name in enumerate(out_names)}
        for c in range(N_CORES)
    ]


def kernel(node_features, w1, b1, w2, b2):
    node_features = np.asarray(node_features, np.float32)
    w1 = np.asarray(w1, np.float32)
    b1 = np.asarray(b1, np.float32)
    w2 = np.asarray(w2, np.float32)
    b2 = np.asarray(b2, np.float32)
    in_maps = _host_prep(node_features, w1, b1, w2, b2)
    results = _run(in_maps)
    return _gather(results)



# revision 23
# speedup vs baseline: 1.3841x; 1.3841x over previous
"""Bass/Trainium2 kernel for nn_DynamicEdgeWeights.

Math (B=4, S=512, D=128, H=8):
    a = x @ w1[:D]; c = x @ w1[D:]
    h[b,i,j,:] = relu(a[b,i,:] + c[b,j,:] + b1)
    out[b,h,i,j] = sigmoid(sum_d h[b,i,j,d] * w2[d,h] + b2[h])

Device strategy (per core; 8 cores, core k -> batch k//2, i-rows [(k%2)*256, +256)):
  - cT[d, j] = (x[b] @ w1c).T and aT[d, i] = (x[b] @ w1a).T + b1 via two PE
    matmuls on pre-transposed x (host passes x[b].T).
  - per query row i: one fused relu(cT + aT[:, i]) producing h_i [128d, 512j]
    (DVE tensor_scalar add+max, or ACT activation Relu with per-partition bias).
  - second matmul uses "comb" weights: 16 query rows share one PSUM bank.
    comb_g [128, 128] has w2[:, h] in column h*16+g, zeros elsewhere; 16
    accumulating matmuls put e-pre for (16 i x 8 h) on 128 PSUM partitions.
  - groups are processed in pairs sharing a 2-bank PSUM tile; one full-width
    [128, 1024] sigmoid (ACT, bias=b2 broadcast) -> SBUF -> stores split
    across both HWDGE queues straight into out[b, :, i-rows, :].
"""

import os
import sys

for _p in ("/opt/trn_rl_repo", "/root/.axon_site/_ro/trn_rl_repo"):
    if os.path.isdir(_p) and _p not in sys.path:
        sys.path.insert(0, _p)
        break

import numpy as np
import ml_dtypes  # noqa: F401  (registers bfloat16 dtype)

import concourse.bass as bass  # noqa: F401  (registers types)
import concourse.mybir as mybir
from concourse import bacc
from concourse.tile import TileContext

B, S, D, H = 4, 512, 128, 8
N_CORES = 8
I_PER_CORE = (B * S) // N_CORES  # 256
G = 16  # query rows packed per PSUM bank
T = I_PER_CORE // G  # 16 groups per core
# ACT h-gen share: values >= 8 mean (n_act - 8) of the last 8 sched entries
# per pair go to ScalarE (rest DVE); 13 -> 5 of 32 rows per pair on ACT
N_ACT = 13

F32 = mybir.dt.float32
F16 = mybir.dt.float16  # h-path dtype: full PE rate (fp32 streams at 1/4 rate)
F8 = mybir.dt.float8e4  # optional ACT-row dtype (ACT writes 1-byte faster)
F16_NP = "float16"

_CACHE: dict = {}


def _build_nc(loop_iters: int = 1, dt_h=F16, n_act=N_ACT, h_bufs=8, o_bufs=8, mm_bufs=4, staggered=False, diag=None, store_eng="one2a", mm_order="rqu", act_f8=False):
    """Build the single-core Bass program (identical across the 8 cores).

    loop_iters > 1 wraps the whole compute in an on-device For_i repeat —
    used only for steady-state timing (one dispatch, N executions).
    """
    nc = bacc.Bacc(
        "TRN2",
        target_bir_lowering=False,
        debug=False,
        enable_asserts=False,
        num_devices=N_CORES,
    )

    xTj_d = nc.dram_tensor("xTj", (D, S), F32, kind="ExternalInput").ap()
    xTi_d = nc.dram_tensor("xTi", (D, I_PER_CORE), F32, kind="ExternalInput").ap()
    w1a_d = nc.dram_tensor("w1a", (D, D), F32, kind="ExternalInput").ap()
    w1c_d = nc.dram_tensor("w1c", (D, D), F32, kind="ExternalInput").ap()
    b1_d = nc.dram_tensor("b1c", (D, 1), F32, kind="ExternalInput").ap()
    comb_d = nc.dram_tensor("comb", (D, 4 * 32), dt_h, kind="ExternalInput").ap()
    b2v_d = nc.dram_tensor("b2v", (D, 1), F32, kind="ExternalInput").ap()
    if store_eng.startswith("one2"):
        # raw drain-major layout: [drain, (g,h) partition, (u,j)]; host
        # un-permutes in _gather. Stores are fully contiguous 256KB.
        out_d = nc.dram_tensor(
            "out", (T // 2, D, 2 * S), F32, kind="ExternalOutput"
        ).ap()
    else:
        out_d = nc.dram_tensor(
            "out", (H, I_PER_CORE, S), F32, kind="ExternalOutput"
        ).ap()

    relu = mybir.ActivationFunctionType.Relu
    sigmoid = mybir.ActivationFunctionType.Sigmoid
    add = mybir.AluOpType.add
    amax = mybir.AluOpType.max

    import contextlib

    with TileContext(nc) as tc:
        with (
            tc.tile_pool(name="const", bufs=1) as cpool,
            tc.tile_pool(name="h", bufs=h_bufs) as hpool,
            tc.tile_pool(name="o", bufs=o_bufs) as opool,
            tc.tile_pool(name="mm", bufs=mm_bufs, space="PSUM") as mmpool,
            (
                tc.For_i(
                    0,
                    loop_iters,
                    1,
                    hint_engines=(
                        mybir.EngineType.PE,
                        mybir.EngineType.DVE,
                        mybir.EngineType.Activation,
                        mybir.EngineType.SP,
                    ),
                    staggered_reset=staggered,
                )
                if loop_iters > 1
                else contextlib.nullcontext()
            ),
        ):
            xj_sb = cpool.tile([D, S], F32)
            nc.sync.dma_start(out=xj_sb, in_=xTj_d)
            xi_sb = cpool.tile([D, I_PER_CORE], F32)
            nc.sync.dma_start(out=xi_sb, in_=xTi_d)
            w1a_sb = cpool.tile([D, D], F32)
            nc.sync.dma_start(out=w1a_sb, in_=w1a_d)
            w1c_sb = cpool.tile([D, D], F32)
            nc.sync.dma_start(out=w1c_sb, in_=w1c_d)
            comb_sb = cpool.tile([D, 4 * 32], dt_h)
            nc.scalar.dma_start(out=comb_sb, in_=comb_d)
            b1_sb = cpool.tile([D, 1], F32)
            nc.sync.dma_start(out=b1_sb, in_=b1_d)
            b2v_sb = cpool.tile([D, 1], F32)
            nc.sync.dma_start(out=b2v_sb, in_=b2v_d)

            # precompute borrows one pair-slot from the matmul psum pool:
            # cT in the first bank-half, aT in the second
            pre_ps = mmpool.tile([D, 2 * S], F32, tag="ps2")
            # cT[d_out, j] = sum_k w1c[k, d_out] * xT[k, j]
            nc.tensor.matmul(pre_ps[:, :S], w1c_sb, xj_sb, start=True, stop=True)
            cT_sb = cpool.tile([D, S], dt_h)
            nc.vector.tensor_copy(cT_sb, pre_ps[:, :S])

            # aT[d_out, i] = sum_k w1a[k, d_out] * xT[k, i]  (+ b1 per partition)
            nc.tensor.matmul(
                pre_ps[:, S : S + I_PER_CORE], w1a_sb, xi_sb, start=True, stop=True
            )
            at_sb = cpool.tile([D, I_PER_CORE], F32)
            nc.vector.tensor_scalar_add(at_sb, pre_ps[:, S : S + I_PER_CORE], b1_sb)

            def drain(t, ps2):
                # sigmoid + store for a finished pair of groups (t, t+1);
                # emitted one pair late so ACT's (stalling) sigmoid sits
                # behind the next pair's h-gen ops in ACT program order.
                o_sb = opool.tile([D, 2 * S], F32)
                nc.scalar.activation(o_sb, ps2, sigmoid, bias=b2v_sb)
                if store_eng.startswith("one2"):
                    # single fully-contiguous 256KB store per drain
                    if store_eng == "one2a":
                        eng = nc.scalar
                    elif store_eng == "one2s":
                        eng = nc.sync
                    else:  # one2: alternate queues per drain
                        eng = nc.sync if (t // 2) % 2 == 0 else nc.scalar
                    eng.dma_start(out=out_d[t // 2], in_=o_sb)
                    return
                # partition p = g*8+h  ->  out[h, (t+u)*16+g, :]
                half = D // 2
                for u in range(2):
                    dst = out_d[:, (t + u) * G : (t + u + 1) * G, :].rearrange(
                        "h g j -> g h j"
                    )
                    src = o_sb[:, u * S : (u + 1) * S]
                    if store_eng == "big":
                        # one full-width store per u-half; alternate engines
                        eng = nc.sync if u == 0 else nc.scalar
                        eng.dma_start(out=dst, in_=src)
                    elif store_eng == "bigsp":
                        nc.sync.dma_start(out=dst, in_=src)
                    else:  # "split": halves across SP + ACT queues
                        nc.sync.dma_start(out=dst[: G // 2], in_=src[:half])
                        nc.scalar.dma_start(out=dst[G // 2 :], in_=src[half:])

            if diag == "pe":
                # PE-pure stream: one static h tile, full matmul schedule
                h_static = cpool.tile([D, 2 * S], dt_h)
                nc.vector.tensor_copy(h_static[:, :S], cT_sb)
                nc.vector.tensor_copy(h_static[:, S:], cT_sb)
                for t in range(0, T, 2):
                    ps2 = mmpool.tile([D, 2 * S], F32, tag="ps2")
                    g_order = [4 * q + r for r in range(4) for q in range(4)]
                    for n, g in enumerate(g_order):
                        q, r = g // 4, g % 4
                        for u in range(2):
                            nc.tensor.matmul(
                                ps2[32 * q : 32 * (q + 1), u * S : (u + 1) * S],
                                comb_sb[:, 32 * r : 32 * (r + 1)],
                                h_static[:, u * S : (u + 1) * S],
                                start=(r == 0),
                                stop=(r == 3),
                                tile_position=(0, 32 * q),
                                skip_group_check=True,
                            )
                    o_sb = opool.tile([D, 2 * S], F32)
                    nc.scalar.activation(o_sb, ps2, sigmoid, bias=b2v_sb)
                    half = D // 2
                    for u in range(2):
                        dst = out_d[:, (t + u) * G : (t + u + 1) * G, :]
                        src = o_sb[:, u * S : (u + 1) * S]
                        nc.sync.dma_start(out=dst[: H // 2], in_=src[:half])
                        nc.scalar.dma_start(out=dst[H // 2 :], in_=src[half:])
            elif diag == "dve":
                # DVE-pure stream: all h-gen ops, no matmul/sigmoid; dump one
                # h tile to out to keep outputs written
                for t in range(0, T, 2):
                    for g in range(G):
                        h2 = hpool.tile([D, 2 * S], dt_h)
                        for u in range(2):
                            i_loc = (t + u) * G + g
                            a_col = at_sb[:, i_loc : i_loc + 1]
                            dst = h2[:, u * S : (u + 1) * S]
                            nc.vector.tensor_scalar(dst, cT_sb, a_col, 0.0, add, amax)
                    o_sb = opool.tile([D, 2 * S], F32)
                    nc.vector.tensor_copy(o_sb, h2)
                    half = D // 2
                    for u in range(2):
                        dst = out_d[:, (t + u) * G : (t + u + 1) * G, :]
                        src = o_sb[:, u * S : (u + 1) * S]
                        nc.sync.dma_start(out=dst[: H // 2], in_=src[:half])
                        nc.scalar.dma_start(out=dst[H // 2 :], in_=src[half:])
            else:
                pending = None  # (t, psum tile) awaiting sigmoid+store
                # emission order: r outer, u middle, q fastest -> consecutive
                # matmuls land in 4 different 32-col PE strips (concurrent
                # streaming); a strip's accumulation chain (same q,u across r)
                # recurs only every 8 instructions.
                if mm_order == "ruq":
                    sched = [
                        (4 * q + r, u)
                        for r in range(4)
                        for u in range(2)
                        for q in range(4)
                    ]
                else:  # "rqu": the original order, u innermost
                    sched = [
                        (4 * q + r, u)
                        for r in range(4)
                        for q in range(4)
                        for u in range(2)
                    ]
                for t in range(0, T, 2):
                    # two groups (t, t+1) share one 2-bank PSUM tile: matmul g
                    # covers j 0..511 for row t*16+g and j 512..1023 for row
                    # (t+1)*16+g with the same comb_g weights.
                    ps2 = mmpool.tile([D, 2 * S], F32, tag="ps2")
                    for n, (g, u) in enumerate(sched):
                        q, r = g // 4, g % 4
                        i_loc = (t + u) * G + g
                        a_col = at_sb[:, i_loc : i_loc + 1]
                        # last act_k of the 32 sched entries go to ACT (all
                        # r==3 tail positions when act_k <= 8)
                        act_k = n_act * 2 if n_act < 8 else n_act - 8
                        on_act = n >= 32 - act_k
                        dt_row = F8 if (on_act and act_f8) else dt_h
                        hu = hpool.tile([D, S], dt_row, tag=f"h{u}{'f8' if dt_row is F8 else ''}")
                        if on_act:
                            nc.scalar.activation(hu, cT_sb, relu, bias=a_col)
                        else:
                            nc.vector.tensor_scalar(hu, cT_sb, a_col, 0.0, add, amax)
                        nc.tensor.matmul(
                            ps2[32 * q : 32 * (q + 1), u * S : (u + 1) * S],
                            comb_sb[:, 32 * r : 32 * (r + 1)],
                            hu,
                            start=(r == 0),
                            stop=(r == 3),
                            tile_position=(0, 32 * q),
                            skip_group_check=True,
                        )
                        if n == 3 and pending is not None:
                            drain(*pending)
                            pending = None
                    pending = (t, ps2)
                drain(*pending)

    nc.compile()
    return nc


def _host_prep(node_features, w1, b1, w2, b2):
    """Shared (per-core-replicated) small tensors + per-core input maps."""
    w1a = np.ascontiguousarray(w1[:D])  # [D, D] == lhsT for aT
    w1c = np.ascontiguousarray(w1[D:])  # [D, D] == lhsT for cT
    b1c = np.ascontiguousarray(b1.reshape(D, 1))
    # psum partition p = g*8 + h; col-group q = g//4 covers partitions
    # [32q, 32q+32); weight tile r = g%4 has w2 in columns [8r, 8r+8)
    comb = np.zeros((D, 4, 32), np.float32)
    for r in range(4):
        comb[:, r, r * H : (r + 1) * H] = w2
    comb = np.ascontiguousarray(comb.reshape(D, 4 * 32).astype(F16_NP))
    b2v = np.ascontiguousarray(np.tile(b2, G).reshape(D, 1))

    in_maps = []
    for k in range(N_CORES):
        b = k // (N_CORES // B)
        i0 = (k % (N_CORES // B)) * I_PER_CORE
        xT = np.ascontiguousarray(node_features[b].T)  # [D, S]
        in_maps.append(
            {
                "xTj": xT,
                "xTi": np.ascontiguousarray(xT[:, i0 : i0 + I_PER_CORE]),
                "w1a": w1a,
                "w1c": w1c,
                "b1c": b1c,
                "comb": comb,
                "b2v": b2v,
            }
        )
    return in_maps


def _gather(results):
    out = np.empty((B, H, S, S), np.float32)
    for k in range(N_CORES):
        b = k // (N_CORES // B)
        i0 = (k % (N_CORES // B)) * I_PER_CORE
        arr = results[k]["out"]
        if arr.shape[0] == T // 2:  # raw drain-major layout (one2 stores)
            # arr[d, g*8+h, u*512+j] -> out[b, h, i0 + d*32+u*16+g, j]
            a5 = arr.reshape(T // 2, G, H, 2, S)  # [d, g, h, u, j]
            out[b, :, i0 : i0 + I_PER_CORE, :] = (
                a5.transpose(2, 0, 3, 1, 4).reshape(H, I_PER_CORE, S)
            )
        else:
            out[b, :, i0 : i0 + I_PER_CORE, :] = arr
    return out


def _build_jit(nc):
    """Single cached jit around the bass_exec custom call (the stock
    run_bass_kernel_spmd path re-traces/jits on every invocation)."""
    import jax
    from jax.sharding import Mesh, PartitionSpec

    try:
        from jax.experimental.shard_map import shard_map
    except ImportError:
        from jax.sharding import shard_map

    from concourse.bass2jax import (
        _bass_exec_p,
        install_neuronx_cc_hook,
        partition_id_tensor,
    )

    install_neuronx_cc_hook()
    partition_name = nc.partition_id_tensor.name if nc.partition_id_tensor else None
    in_names, out_names, out_avals, zero_outs = [], [], [], []
    for alloc in nc.m.functions[0].allocations:
        if not isinstance(alloc, mybir.MemoryLocationSet):
            continue
        name = alloc.memorylocations[0].name
        if alloc.kind == "ExternalInput":
            if name != partition_name:
                in_names.append(name)
        elif alloc.kind == "ExternalOutput":
            shape = tuple(alloc.tensor_shape)
            np_dt = mybir.dt.np(alloc.dtype)
            out_avals.append(jax.core.ShapedArray(shape, np_dt))
            out_names.append(name)
            zero_outs.append(np.zeros(shape, np_dt))
    n_params = len(in_names)
    all_in_names = list(in_names) + list(out_names)
    if partition_name is not None:
        all_in_names.append(partition_name)

    def _body(*args):
        operands = list(args)
        if partition_name is not None:
            operands.append(partition_id_tensor())
        return tuple(
            _bass_exec_p.bind(
                *operands,
                out_avals=tuple(out_avals),
                in_names=tuple(all_in_names),
                out_names=tuple(out_names),
                lowering_input_output_aliases=(),
                sim_require_finite=True,
                sim_require_nnan=True,
                nc=nc,
            )
        )

    devices = jax.devices()[:N_CORES]
    mesh = Mesh(np.asarray(devices), ("core",))
    n_outs = len(out_names)
    sharded = jax.jit(
        shard_map(
            _body,
            mesh=mesh,
            in_specs=(PartitionSpec("core"),) * (n_params + n_outs),
            out_specs=(PartitionSpec("core"),) * n_outs,
            check_rep=False,
        ),
        # no donation: the kernel writes every output element, so the zero
        # operand buffers can live on device and be reused across calls
        keep_unused=True,
    )
    return sharded, in_names, out_names, zero_outs


def _run(in_maps):
    if "nc" not in _CACHE:
        _CACHE["nc"] = _build_nc()
        _CACHE["jit"] = _build_jit(_CACHE["nc"])
    sharded, in_names, out_names, zero_outs = _CACHE["jit"]
    concat_in = [
        np.concatenate([np.asarray(in_maps[c][n]) for c in range(N_CORES)], axis=0)
        for n in in_names
    ]
    if "zeros_dev" not in _CACHE:
        import jax

        _CACHE["zeros_dev"] = [
            jax.device_put(np.zeros((N_CORES * z.shape[0], *z.shape[1:]), z.dtype))
            for z in zero_outs
        ]
    out_arrs = sharded(*concat_in, *_CACHE["zeros_dev"])
    # outputs come back concatenated on axis 0 (N_CORES * dim0, ...)
    split = []
    for i, name in enumerate(out_names):
        arr = np.asarray(out_arrs[i])
        split.append(arr.reshape(N_CORES, arr.shape[0] // N_CORES, *arr.shape[1:]))
    return [
        {name: split[i][c] for i, name in enumerate(out_names)}
        for c in range(N_CORES)
    ]


def kernel(node_features, w1, b1, w2, b2):
    node_features = np.asarray(node_features, np.float32)
    w1 = np.asarray(w1, np.float32)
    b1 = np.asarray(b1, np.float32)
    w2 = np.asarray(w2, np.float32)
    b2 = np.asarray(b2, np.float32)
    in_maps = _host_prep(node_features, w1, b1, w2, b2)
    results = _run(in_maps)
    return _gather(results)



# revision 27
# speedup vs baseline: 1.4615x; 1.0559x over previous
"""Bass/Trainium2 kernel for nn_DynamicEdgeWeights.

Math (B=4, S=512, D=128, H=8):
    a = x @ w1[:D]; c = x @ w1[D:]
    h[b,i,j,:] = relu(a[b,i,:] + c[b,j,:] + b1)
    out[b,h,i,j] = sigmoid(sum_d h[b,i,j,d] * w2[d,h] + b2[h])

Device strategy (per core; 8 cores, core k -> batch k//2, i-rows [(k%2)*256, +256)):
  - cT[d, j] = (x[b] @ w1c).T and aT[d, i] = (x[b] @ w1a).T + b1 via two PE
    matmuls on pre-transposed x (host passes x[b].T).
  - per query row i: one fused relu(cT + aT[:, i]) producing h_i [128d, 512j]
    (DVE tensor_scalar add+max, or ACT activation Relu with per-partition bias).
  - second matmul uses "comb" weights: 16 query rows share one PSUM bank.
    comb_g [128, 128] has w2[:, h] in column h*16+g, zeros elsewhere; 16
    accumulating matmuls put e-pre for (16 i x 8 h) on 128 PSUM partitions.
  - groups are processed in pairs sharing a 2-bank PSUM tile; one full-width
    [128, 1024] sigmoid (ACT, bias=b2 broadcast) -> SBUF -> stores split
    across both HWDGE queues straight into out[b, :, i-rows, :].
"""

import os
import sys

for _p in ("/opt/trn_rl_repo", "/root/.axon_site/_ro/trn_rl_repo"):
    if os.path.isdir(_p) and _p not in sys.path:
        sys.path.insert(0, _p)
        break

import numpy as np
import ml_dtypes  # noqa: F401  (registers bfloat16 dtype)

import concourse.bass as bass  # noqa: F401  (registers types)
import concourse.mybir as mybir
from concourse import bacc
from concourse.tile import TileContext

B, S, D, H = 4, 512, 128, 8
N_CORES = 8
I_PER_CORE = (B * S) // N_CORES  # 256
G = 16  # query rows packed per PSUM bank
T = I_PER_CORE // G  # 16 groups per core
# ACT h-gen share: values >= 8 mean (n_act - 8) of the last 8 sched entries
# per pair go to ScalarE (rest DVE); 13 -> 5 of 32 rows per pair on ACT
N_ACT = 13

F32 = mybir.dt.float32
F16 = mybir.dt.float16  # h-path dtype: full PE rate (fp32 streams at 1/4 rate)
F8 = mybir.dt.float8e4  # optional ACT-row dtype (ACT writes 1-byte faster)
F16_NP = "float16"

_CACHE: dict = {}


def _build_nc(loop_iters: int = 1, dt_h=F16, n_act=N_ACT, h_bufs=8, o_bufs=8, mm_bufs=4, staggered=False, diag=None, store_eng="one2a", mm_order="rqu", act_f8=False, store_f16=False):
    """Build the single-core Bass program (identical across the 8 cores).

    loop_iters > 1 wraps the whole compute in an on-device For_i repeat —
    used only for steady-state timing (one dispatch, N executions).
    """
    nc = bacc.Bacc(
        "TRN2",
        target_bir_lowering=False,
        debug=False,
        enable_asserts=False,
        num_devices=N_CORES,
    )

    xTj_d = nc.dram_tensor("xTj", (D, S), F32, kind="ExternalInput").ap()
    xTi_d = nc.dram_tensor("xTi", (D, I_PER_CORE), F32, kind="ExternalInput").ap()
    w1a_d = nc.dram_tensor("w1a", (D, D), F32, kind="ExternalInput").ap()
    w1c_d = nc.dram_tensor("w1c", (D, D), F32, kind="ExternalInput").ap()
    b1_d = nc.dram_tensor("b1c", (D, 1), F32, kind="ExternalInput").ap()
    comb_d = nc.dram_tensor("comb", (D, 4 * 32), dt_h, kind="ExternalInput").ap()
    b2v_d = nc.dram_tensor("b2v", (D, 1), F32, kind="ExternalInput").ap()
    dt_o = F16 if store_f16 else F32
    if store_eng.startswith("one2"):
        # raw drain-major layout: [drain, (g,h) partition, (u,j)]; host
        # un-permutes in _gather. Stores are fully contiguous.
        out_d = nc.dram_tensor(
            "out", (T // 2, D, 2 * S), dt_o, kind="ExternalOutput"
        ).ap()
    else:
        out_d = nc.dram_tensor(
            "out", (H, I_PER_CORE, S), dt_o, kind="ExternalOutput"
        ).ap()

    relu = mybir.ActivationFunctionType.Relu
    sigmoid = mybir.ActivationFunctionType.Sigmoid
    add = mybir.AluOpType.add
    amax = mybir.AluOpType.max

    import contextlib

    with TileContext(nc) as tc:
        with (
            tc.tile_pool(name="const", bufs=1) as cpool,
            tc.tile_pool(name="h", bufs=h_bufs) as hpool,
            tc.tile_pool(name="o", bufs=o_bufs) as opool,
            tc.tile_pool(name="mm", bufs=mm_bufs, space="PSUM") as mmpool,
            (
                tc.For_i(
                    0,
                    loop_iters,
                    1,
                    hint_engines=(
                        mybir.EngineType.PE,
                        mybir.EngineType.DVE,
                        mybir.EngineType.Activation,
                        mybir.EngineType.SP,
                    ),
                    staggered_reset=staggered,
                )
                if loop_iters > 1
                else contextlib.nullcontext()
            ),
        ):
            xj_sb = cpool.tile([D, S], F32)
            nc.sync.dma_start(out=xj_sb, in_=xTj_d)
            xi_sb = cpool.tile([D, I_PER_CORE], F32)
            nc.sync.dma_start(out=xi_sb, in_=xTi_d)
            w1a_sb = cpool.tile([D, D], F32)
            nc.sync.dma_start(out=w1a_sb, in_=w1a_d)
            w1c_sb = cpool.tile([D, D], F32)
            nc.sync.dma_start(out=w1c_sb, in_=w1c_d)
            comb_sb = cpool.tile([D, 4 * 32], dt_h)
            nc.scalar.dma_start(out=comb_sb, in_=comb_d)
            b1_sb = cpool.tile([D, 1], F32)
            nc.sync.dma_start(out=b1_sb, in_=b1_d)
            b2v_sb = cpool.tile([D, 1], F32)
            nc.sync.dma_start(out=b2v_sb, in_=b2v_d)

            # precompute borrows one pair-slot from the matmul psum pool:
            # cT in the first bank-half, aT in the second
            pre_ps = mmpool.tile([D, 2 * S], F32, tag="ps2")
            # cT[d_out, j] = sum_k w1c[k, d_out] * xT[k, j]
            nc.tensor.matmul(pre_ps[:, :S], w1c_sb, xj_sb, start=True, stop=True)
            cT_sb = cpool.tile([D, S], dt_h)
            nc.vector.tensor_copy(cT_sb, pre_ps[:, :S])

            # aT[d_out, i] = sum_k w1a[k, d_out] * xT[k, i]  (+ b1 per partition)
            nc.tensor.matmul(
                pre_ps[:, S : S + I_PER_CORE], w1a_sb, xi_sb, start=True, stop=True
            )
            at_sb = cpool.tile([D, I_PER_CORE], F32)
            nc.vector.tensor_scalar_add(at_sb, pre_ps[:, S : S + I_PER_CORE], b1_sb)

            def drain(t, ps2):
                # sigmoid + store for a finished pair of groups (t, t+1);
                # emitted one pair late so ACT's (stalling) sigmoid sits
                # behind the next pair's h-gen ops in ACT program order.
                o_sb = opool.tile([D, 2 * S], dt_o)
                nc.scalar.activation(o_sb, ps2, sigmoid, bias=b2v_sb)
                if store_eng.startswith("one2"):
                    # single fully-contiguous 256KB store per drain
                    if store_eng == "one2a":
                        eng = nc.scalar
                    elif store_eng == "one2s":
                        eng = nc.sync
                    else:  # one2: alternate queues per drain
                        eng = nc.sync if (t // 2) % 2 == 0 else nc.scalar
                    eng.dma_start(out=out_d[t // 2], in_=o_sb)
                    return
                # partition p = g*8+h  ->  out[h, (t+u)*16+g, :]
                half = D // 2
                for u in range(2):
                    dst = out_d[:, (t + u) * G : (t + u + 1) * G, :].rearrange(
                        "h g j -> g h j"
                    )
                    src = o_sb[:, u * S : (u + 1) * S]
                    if store_eng == "big":
                        # one full-width store per u-half; alternate engines
                        eng = nc.sync if u == 0 else nc.scalar
                        eng.dma_start(out=dst, in_=src)
                    elif store_eng == "bigsp":
                        nc.sync.dma_start(out=dst, in_=src)
                    else:  # "split": halves across SP + ACT queues
                        nc.sync.dma_start(out=dst[: G // 2], in_=src[:half])
                        nc.scalar.dma_start(out=dst[G // 2 :], in_=src[half:])

            if diag == "pe":
                # PE-pure stream: one static h tile, full matmul schedule
                h_static = cpool.tile([D, 2 * S], dt_h)
                nc.vector.tensor_copy(h_static[:, :S], cT_sb)
                nc.vector.tensor_copy(h_static[:, S:], cT_sb)
                for t in range(0, T, 2):
                    ps2 = mmpool.tile([D, 2 * S], F32, tag="ps2")
                    g_order = [4 * q + r for r in range(4) for q in range(4)]
                    for n, g in enumerate(g_order):
                        q, r = g // 4, g % 4
                        for u in range(2):
                            nc.tensor.matmul(
                                ps2[32 * q : 32 * (q + 1), u * S : (u + 1) * S],
                                comb_sb[:, 32 * r : 32 * (r + 1)],
                                h_static[:, u * S : (u + 1) * S],
                                start=(r == 0),
                                stop=(r == 3),
                                tile_position=(0, 32 * q),
                                skip_group_check=True,
                            )
                    o_sb = opool.tile([D, 2 * S], F32)
                    nc.scalar.activation(o_sb, ps2, sigmoid, bias=b2v_sb)
                    half = D // 2
                    for u in range(2):
                        dst = out_d[:, (t + u) * G : (t + u + 1) * G, :]
                        src = o_sb[:, u * S : (u + 1) * S]
                        nc.sync.dma_start(out=dst[: H // 2], in_=src[:half])
                        nc.scalar.dma_start(out=dst[H // 2 :], in_=src[half:])
            elif diag == "dve":
                # DVE-pure stream: all h-gen ops, no matmul/sigmoid; dump one
                # h tile to out to keep outputs written
                for t in range(0, T, 2):
                    for g in range(G):
                        h2 = hpool.tile([D, 2 * S], dt_h)
                        for u in range(2):
                            i_loc = (t + u) * G + g
                            a_col = at_sb[:, i_loc : i_loc + 1]
                            dst = h2[:, u * S : (u + 1) * S]
                            nc.vector.tensor_scalar(dst, cT_sb, a_col, 0.0, add, amax)
                    o_sb = opool.tile([D, 2 * S], F32)
                    nc.vector.tensor_copy(o_sb, h2)
                    half = D // 2
                    for u in range(2):
                        dst = out_d[:, (t + u) * G : (t + u + 1) * G, :]
                        src = o_sb[:, u * S : (u + 1) * S]
                        nc.sync.dma_start(out=dst[: H // 2], in_=src[:half])
                        nc.scalar.dma_start(out=dst[H // 2 :], in_=src[half:])
            else:
                pending = None  # (t, psum tile) awaiting sigmoid+store
                # emission order: r outer, u middle, q fastest -> consecutive
                # matmuls land in 4 different 32-col PE strips (concurrent
                # streaming); a strip's accumulation chain (same q,u across r)
                # recurs only every 8 instructions.
                if mm_order == "ruq":
                    sched = [
                        (4 * q + r, u)
                        for r in range(4)
                        for u in range(2)
                        for q in range(4)
                    ]
                else:  # "rqu": the original order, u innermost
                    sched = [
                        (4 * q + r, u)
                        for r in range(4)
                        for q in range(4)
                        for u in range(2)
                    ]
                for t in range(0, T, 2):
                    # two groups (t, t+1) share one 2-bank PSUM tile: matmul g
                    # covers j 0..511 for row t*16+g and j 512..1023 for row
                    # (t+1)*16+g with the same comb_g weights.
                    ps2 = mmpool.tile([D, 2 * S], F32, tag="ps2")
                    for n, (g, u) in enumerate(sched):
                        q, r = g // 4, g % 4
                        i_loc = (t + u) * G + g
                        a_col = at_sb[:, i_loc : i_loc + 1]
                        # last act_k of the 32 sched entries go to ACT (all
                        # r==3 tail positions when act_k <= 8)
                        act_k = n_act * 2 if n_act < 8 else n_act - 8
                        on_act = n >= 32 - act_k
                        dt_row = F8 if (on_act and act_f8) else dt_h
                        hu = hpool.tile([D, S], dt_row, tag=f"h{u}{'f8' if dt_row is F8 else ''}")
                        if on_act:
                            nc.scalar.activation(hu, cT_sb, relu, bias=a_col)
                        else:
                            nc.vector.tensor_scalar(hu, cT_sb, a_col, 0.0, add, amax)
                        nc.tensor.matmul(
                            ps2[32 * q : 32 * (q + 1), u * S : (u + 1) * S],
                            comb_sb[:, 32 * r : 32 * (r + 1)],
                            hu,
                            start=(r == 0),
                            stop=(r == 3),
                            tile_position=(0, 32 * q),
                            skip_group_check=True,
                        )
                        if n == 3 and pending is not None:
                            drain(*pending)
                            pending = None
                    pending = (t, ps2)
                drain(*pending)

    nc.compile()
    return nc


def _host_prep(node_features, w1, b1, w2, b2):
    """Shared (per-core-replicated) small tensors + per-core input maps."""
    w1a = np.ascontiguousarray(w1[:D])  # [D, D] == lhsT for aT
    w1c = np.ascontiguousarray(w1[D:])  # [D, D] == lhsT for cT
    b1c = np.ascontiguousarray(b1.reshape(D, 1))
    # psum partition p = g*8 + h; col-group q = g//4 covers partitions
    # [32q, 32q+32); weight tile r = g%4 has w2 in columns [8r, 8r+8)
    comb = np.zeros((D, 4, 32), np.float32)
    for r in range(4):
        comb[:, r, r * H : (r + 1) * H] = w2
    comb = np.ascontiguousarray(comb.reshape(D, 4 * 32).astype(F16_NP))
    b2v = np.ascontiguousarray(np.tile(b2, G).reshape(D, 1))

    in_maps = []
    for k in range(N_CORES):
        b = k // (N_CORES // B)
        i0 = (k % (N_CORES // B)) * I_PER_CORE
        xT = np.ascontiguousarray(node_features[b].T)  # [D, S]
        in_maps.append(
            {
                "xTj": xT,
                "xTi": np.ascontiguousarray(xT[:, i0 : i0 + I_PER_CORE]),
                "w1a": w1a,
                "w1c": w1c,
                "b1c": b1c,
                "comb": comb,
                "b2v": b2v,
            }
        )
    return in_maps


def _gather(results):
    out = np.empty((B, H, S, S), np.float32)
    for k in range(N_CORES):
        b = k // (N_CORES // B)
        i0 = (k % (N_CORES // B)) * I_PER_CORE
        arr = results[k]["out"]
        if arr.dtype != np.float32:
            arr = arr.astype(np.float32)
        if arr.shape[0] == T // 2:  # raw drain-major layout (one2 stores)
            # arr[d, g*8+h, u*512+j] -> out[b, h, i0 + d*32+u*16+g, j]
            a5 = arr.reshape(T // 2, G, H, 2, S)  # [d, g, h, u, j]
            out[b, :, i0 : i0 + I_PER_CORE, :] = (
                a5.transpose(2, 0, 3, 1, 4).reshape(H, I_PER_CORE, S)
            )
        else:
            out[b, :, i0 : i0 + I_PER_CORE, :] = arr
    return out


def _build_jit(nc):
    """Single cached jit around the bass_exec custom call (the stock
    run_bass_kernel_spmd path re-traces/jits on every invocation)."""
    import jax
    from jax.sharding import Mesh, PartitionSpec

    try:
        from jax.experimental.shard_map import shard_map
    except ImportError:
        from jax.sharding import shard_map

    from concourse.bass2jax import (
        _bass_exec_p,
        install_neuronx_cc_hook,
        partition_id_tensor,
    )

    install_neuronx_cc_hook()
    partition_name = nc.partition_id_tensor.name if nc.partition_id_tensor else None
    in_names, out_names, out_avals, zero_outs = [], [], [], []
    for alloc in nc.m.functions[0].allocations:
        if not isinstance(alloc, mybir.MemoryLocationSet):
            continue
        name = alloc.memorylocations[0].name
        if alloc.kind == "ExternalInput":
            if name != partition_name:
                in_names.append(name)
        elif alloc.kind == "ExternalOutput":
            shape = tuple(alloc.tensor_shape)
            np_dt = mybir.dt.np(alloc.dtype)
            out_avals.append(jax.core.ShapedArray(shape, np_dt))
            out_names.append(name)
            zero_outs.append(np.zeros(shape, np_dt))
    n_params = len(in_names)
    all_in_names = list(in_names) + list(out_names)
    if partition_name is not None:
        all_in_names.append(partition_name)

    def _body(*args):
        operands = list(args)
        if partition_name is not None:
            operands.append(partition_id_tensor())
        return tuple(
            _bass_exec_p.bind(
                *operands,
                out_avals=tuple(out_avals),
                in_names=tuple(all_in_names),
                out_names=tuple(out_names),
                lowering_input_output_aliases=(),
                sim_require_finite=True,
                sim_require_nnan=True,
                nc=nc,
            )
        )

    devices = jax.devices()[:N_CORES]
    mesh = Mesh(np.asarray(devices), ("core",))
    n_outs = len(out_names)
    sharded = jax.jit(
        shard_map(
            _body,
            mesh=mesh,
            in_specs=(PartitionSpec("core"),) * (n_params + n_outs),
            out_specs=(PartitionSpec("core"),) * n_outs,
            check_rep=False,
        ),
        # no donation: the kernel writes every output element, so the zero
        # operand buffers can live on device and be reused across calls
        keep_unused=True,
    )
    return sharded, in_names, out_names, zero_outs


def _run(in_maps):
    if "nc" not in _CACHE:
        _CACHE["nc"] = _build_nc()
        _CACHE["jit"] = _build_jit(_CACHE["nc"])
    sharded, in_names, out_names, zero_outs = _CACHE["jit"]
    concat_in = [
        np.concatenate([np.asarray(in_maps[c][n]) for c in range(N_CORES)], axis=0)
        for n in in_names
    ]
    if "zeros_dev" not in _CACHE:
        import jax

        _CACHE["zeros_dev"] = [
            jax.device_put(np.zeros((N_CORES * z.shape[0], *z.shape[1:]), z.dtype))
            for z in zero_outs
        ]
    out_arrs = sharded(*concat_in, *_CACHE["zeros_dev"])
    # outputs come back concatenated on axis 0 (N_CORES * dim0, ...)
    split = []
    for i, name in enumerate(out_names):
        arr = np.asarray(out_arrs[i])
        split.append(arr.reshape(N_CORES, arr.shape[0] // N_CORES, *arr.shape[1:]))
    return [
        {name: split[i][c] for i, name in enumerate(out_names)}
        for c in range(N_CORES)
    ]


def kernel(node_features, w1, b1, w2, b2):
    node_features = np.asarray(node_features, np.float32)
    w1 = np.asarray(w1, np.float32)
    b1 = np.asarray(b1, np.float32)
    w2 = np.asarray(w2, np.float32)
    b2 = np.asarray(b2, np.float32)
    in_maps = _host_prep(node_features, w1, b1, w2, b2)
    results = _run(in_maps)
    return _gather(results)

